# revision 27
# baseline (speedup 1.0000x reference)
"""Multi-head causal attention (B=4, T=2048, C=1024, H=16) on 8 TRN2 NeuronCores.

Sharding: core c handles batch b=c//2 and head-group g=c%2 (8 heads = 4 pairs).
Per core: QKV projections for its 512 feature columns, causal attention for its
8 heads, partial out-projection. Host sums the two head-group partials per batch
and adds b_o.

All matmul operands are bf16 (f32 PSUM accumulation): 1 cyc/row moving, FWL on
LDWEIGHTS, half the SBUF/DMA of fp32. Softmax normalize: head-B's V' stationary
is padded to 128 cols ([ones | 63 zeros | V_B]) so ctx'B lands on PSUM
partitions 64:127 (denomB at part 0) while ctx'A ([V_A | ones]) lands at 0:64
(denomA at 64); the denominator rows are spread over 64 lanes via gpsimd-queue
DMAs, reciprocated once on DVE (bf16), and a K=33 selector matmul broadcasts
recipA/recipB across psum partitions 0:63/64:127.  The Scalar queue carries
ONLY exp ACTIVATEs (anything else queued there stalls the whole pipeline), and
scores are emitted one block ahead of ctx so the PE computes next-block scores
under the current exp.  Out-projection accumulates pair-groups (0+1, 2+3) in
PSUM, halving the psum-evacuation casts and the output DMA.
"""
import sys
import numpy as np
import ml_dtypes
from contextlib import ExitStack

sys.path.insert(0, "/opt/trn_rl_repo")

import concourse.bass as bass
import concourse.tile as tile
from concourse import bacc, mybir
from concourse.bass_utils import run_bass_kernel_spmd

f32 = mybir.dt.float32
f32r = mybir.dt.float32r
BF = mybir.dt.bfloat16
EXP = mybir.ActivationFunctionType.Exp
LN = mybir.ActivationFunctionType.Ln

C = 1024          # model dim
HG = 512          # per-core head-group feature width (8 heads x 64)
D = 64            # head dim
NPAIR = 4         # head pairs per core
NCC = C // 128    # contraction chunks (8)
SCALE = 0.125     # 1/sqrt(D)
VSLOT = 196       # V' slot: [V_A 0:64 | onesA 64:65 | pad | onesB 68:69 | zeros 69:132 | V_B 132:196]


def build_kernel(T):
    """Emit the per-core Bass program. T = sequence length (multiple of 512)."""
    NQT = T // 512    # q tiles of 512
    NKT = T // 128    # k tiles of 128

    nc = bacc.Bacc("TRN2", target_bir_lowering=False, debug=False, num_devices=8)

    xT = nc.dram_tensor("xT", [C, T], BF, kind="ExternalInput").ap()
    wq = nc.dram_tensor("wq", [C, HG], BF, kind="ExternalInput").ap()
    wk = nc.dram_tensor("wk", [C, HG], BF, kind="ExternalInput").ap()
    wv = nc.dram_tensor("wv", [C, HG], BF, kind="ExternalInput").ap()
    wo = nc.dram_tensor("wo", [HG, C], BF, kind="ExternalInput").ap()
    out = nc.dram_tensor("out", [NPAIR // 2, T, C], BF, kind="ExternalOutput").ap()

    with tile.TileContext(nc) as tc, ExitStack() as ctx:
        # ---- SBUF pools (bytes/partition noted) ----
        p_xt = ctx.enter_context(tc.tile_pool(name="xt", bufs=1))            # 32K
        p_w = ctx.enter_context(tc.tile_pool(name="w", bufs=2))              # 2x2K
        p_wv = ctx.enter_context(tc.tile_pool(name="wv", bufs=1))            # 8K
        p_wo = ctx.enter_context(tc.tile_pool(name="wo", bufs=4))            # 4x2K
        p_qk = ctx.enter_context(tc.tile_pool(name="qk", bufs=4))            # 4x4K=16K
        p_v = ctx.enter_context(tc.tile_pool(name="v", bufs=4))              # 4x6.2K=25K
        p_phat = ctx.enter_context(tc.tile_pool(name="phat", bufs=4))        # 4x2K
        p_ctxT = ctx.enter_context(tc.tile_pool(name="ctxT", bufs=12))       # 12x1K
        p_cxs = ctx.enter_context(tc.tile_pool(name="cxs", bufs=3))          # 3x4K
        p_bc = ctx.enter_context(tc.tile_pool(name="bc", bufs=2))            # 2x2K
        p_small = ctx.enter_context(tc.tile_pool(name="small", bufs=2))      # recips
        p_ostg = ctx.enter_context(tc.tile_pool(name="ostg", bufs=5))        # 5x1K
        p_ones = ctx.enter_context(tc.tile_pool(name="ones", bufs=1))
        # ---- PSUM pools: 4 + 2 + 2 = 8 banks ----
        ps_s = ctx.enter_context(tc.tile_pool(name="ps_s", bufs=2, space="PSUM"))    # [128,1024] x2
        ps_ctx = ctx.enter_context(tc.tile_pool(name="ps_ctx", bufs=1, space="PSUM"))
        ps_mm = ctx.enter_context(tc.tile_pool(name="ps_mm", bufs=2, space="PSUM"))

        # ---- constants + bulk loads ----
        ones_f = p_ones.tile([128, 1], f32)
        nc.vector.memset(ones_f, 1.0)
        # Selector stationary for the reciprocal broadcast: row 0 routes the
        # moving tile's row 0 (recipA) to psum parts 0:63, row 32 routes
        # recipB to parts 64:127; rows 1:31 are zero (matmul K=33).
        ones33 = p_ones.tile([33, 128], BF, tag="ones33")
        nc.vector.memset(ones33, 0.0)
        nc.vector.memset(ones33[0:1, 0:64], 1.0)
        nc.vector.memset(ones33[32:33, 64:128], 1.0)

        # HAM warm-up: ~8us of dummy matmuls during the input DMAs so the
        # PE clock is at 2.4GHz when real work starts.
        warm = p_ostg.tile([128, 512], BF, tag="ostg")
        nc.vector.memset(warm, 0.0)
        wps = ps_mm.tile([128, 512], f32, tag="mm")
        for i in range(28):
            nc.tensor.matmul(wps, warm[:, 0:128], warm,
                             start=(i == 0), stop=(i == 27))

        def load_wqk(p):
            """[128, 8, 128] tile: cc-chunks of W{q,k}[:, p*128:(p+1)*128]."""
            tq = p_w.tile([128, NCC, 128], BF, tag="wq")
            tk = p_w.tile([128, NCC, 128], BF, tag="wk")
            nc.sync.dma_start(
                tq, wq[:, p * 128 : (p + 1) * 128].rearrange("(cc p) f -> p cc f", p=128))
            nc.sync.dma_start(
                tk, wk[:, p * 128 : (p + 1) * 128].rearrange("(cc p) f -> p cc f", p=128))
            return tq, tk

        def load_wo(p):
            t_ = p_wo.tile([128, C], BF, tag="wo")
            nc.sync.dma_start(t_, wo[p * 128 : (p + 1) * 128, :])
            return t_

        # weights first (small), then x in 4 chunk-pair DMAs so the V/QK
        # projection matmuls pipeline behind the arriving chunks; the pair-0
        # weight loads ride idle queues so the x chunks start immediately
        tq0 = p_w.tile([128, NCC, 128], BF, tag="wq")
        tk0 = p_w.tile([128, NCC, 128], BF, tag="wk")
        nc.scalar.dma_start(
            tq0, wq[:, 0:128].rearrange("(cc p) f -> p cc f", p=128))
        nc.gpsimd.dma_start(
            tk0, wk[:, 0:128].rearrange("(cc p) f -> p cc f", p=128))
        w0q, w0k = tq0, tk0
        wv_sb = p_wv.tile([128, NCC, HG], BF)
        nc.scalar.dma_start(wv_sb, wv.rearrange("(cc p) f -> p cc f", p=128))
        xt_all = p_xt.tile([128, NCC, T], BF, tag="xt")
        for i, eng in zip(range(4), (nc.sync, nc.scalar, nc.gpsimd, nc.sync)):
            eng.dma_start(
                xt_all[:, 2 * i : 2 * i + 2, :],
                xT[2 * i * 128 : (2 * i + 2) * 128, :].rearrange(
                    "(cc p) t -> p cc t", p=128))
        xt = [xt_all[:, cc, :] for cc in range(NCC)]

        # ---- filler unit generators (PE work to hide under ACT-bound attention) ----
        v_groups = [None] * (NKT // 4)   # [128, 4, NPAIR, VSLOT] tiles, 4 k-tiles each

        def v_tile(j):
            g = v_groups[j // 4]
            assert g is not None, f"V group {j // 4} not emitted yet"
            return g[:, j % 4]

        v_sb = [None] * NKT

        def v_unit(j):
            def emit():
                ps = ps_mm.tile([128, HG], f32, tag="mm")
                for cc in range(NCC):
                    nc.tensor.matmul(
                        ps, xt[cc][:, j * 128 : (j + 1) * 128],
                        wv_sb[:, cc, :], start=(cc == 0), stop=(cc == NCC - 1))
                if j % 4 == 0:
                    g = p_v.tile([128, 4, NPAIR, VSLOT], BF, tag="v",
                                 name=f"vg{j // 4}")
                    v_groups[j // 4] = g
                    # constant cols for the whole group in 3 strided DVE ops
                    nc.vector.memset(g[:, :, :, 69:132], 0.0)
                    nc.vector.tensor_copy(
                        g[:, :, :, 64:65], ones_f.to_broadcast([128, 4, NPAIR, 1]))
                    nc.vector.tensor_copy(
                        g[:, :, :, 68:69], ones_f.to_broadcast([128, 4, NPAIR, 1]))
                g = v_groups[j // 4]
                # one strided copy scatters all 4 pairs' V_A/V_B blocks:
                # dest free dims [pair(196), head(132), 64]
                dst = bass.AP(
                    tensor=g.tensor, offset=g.offset + (j % 4) * NPAIR * VSLOT,
                    ap=[list(g.ap[0]), [VSLOT, NPAIR], [132, 2], [1, 64]])
                nc.vector.tensor_copy(
                    dst, ps.rearrange("p (a h d) -> p a h d", a=NPAIR, h=2))
                v_sb[j] = v_tile(j)
            return emit

        qkT = {}   # (('q'|'k'), pair) -> [128, T] bf16 tile

        def qk_unit(p, which, wtile, tt):
            def emit():
                key = (which, p)
                if key not in qkT:
                    qkT[key] = p_qk.tile([128, T], BF, tag="qk", name=f"qk_{which}{p}")
                ps = ps_mm.tile([128, 512], f32, tag="mm")
                for cc in range(NCC):
                    nc.tensor.matmul(
                        ps, wtile[:, cc, :], xt[cc][:, tt * 512 : (tt + 1) * 512],
                        start=(cc == 0), stop=(cc == NCC - 1))
                nc.vector.tensor_copy(qkT[key][:, tt * 512 : (tt + 1) * 512], ps)
            return emit

        ctxT_store = {}  # (p, t) -> [128, 512] bf16 tile

        def outproj_unit(g2, t, qq, half, on_act=False):
            def emit():
                stg = p_ostg.tile([128, 512], BF, tag="ostg")
                ps = ps_mm.tile([128, 512], f32, tag="mm")
                for p in (2 * g2, 2 * g2 + 1):
                    ct = ctxT_store[(p, t)]
                    nc.tensor.matmul(
                        ps, ct[:, qq * 128 : (qq + 1) * 128],
                        wo_tiles[p][:, half * 512 : (half + 1) * 512],
                        start=(p == 2 * g2), stop=(p == 2 * g2 + 1))
                if on_act:  # tail only: Scalar is idle once the exps are done
                    nc.scalar.copy(stg, ps)
                else:
                    nc.vector.tensor_copy(stg, ps)
                if qq == 3 and half == 1:
                    ctxT_store.pop((2 * g2, t))
                    ctxT_store.pop((2 * g2 + 1, t))
                nc.sync.dma_start(
                    out[g2, t * 512 + qq * 128 : t * 512 + (qq + 1) * 128,
                        half * 512 : (half + 1) * 512], stg)
            return emit

        pending_norm = []
        reserve = []   # outproj units held back for pair 3's filler-starved start
        wo_tiles = {}

        def make_norm(p, t, cxs):
            ct = p_ctxT.tile([128, 512], BF, tag="ctxT", name=f"ct_{p}_{t}")
            ctxT_store[(p, t)] = ct
            # reciprocal rows now, while the next tile's scores run: spread
            # the two denominator rows over 64 partitions (gpsimd-queue DMAs,
            # keeping Scalar free for exp), one DVE reciprocal, gather the
            # bf16 recips straight into the selector-matmul moving tile.
            rr = p_small.tile([33, 512], BF, tag="rr")
            nc.vector.memset(rr[0:32, :], 0.0)
            sc = p_small.tile([64, 16], f32, tag="sc")
            scb = p_small.tile([64, 16], BF, tag="scb")
            nc.gpsimd.dma_start(sc[0:32, :], cxs[64:65, 0:512])
            nc.gpsimd.dma_start(sc[32:64, :], cxs[0:1, 512:1024])
            with nc.allow_low_precision("softmax 1/denom rows in bf16"):
                nc.vector.reciprocal(scb, sc)
            nc.gpsimd.dma_start(rr[0:1, :], scb[0:32, :])
            nc.gpsimd.dma_start(rr[32:33, :], scb[32:64, :])
            state = {}
            def bcast():
                # PE broadcast: recipA -> psum parts 0:63, recipB -> 64:127
                bc_ps = ps_mm.tile([128, 512], f32, tag="mm")
                nc.tensor.matmul(bc_ps, ones33, rr, start=True, stop=True)
                bc = p_bc.tile([128, 512], f32, tag="bc")
                nc.vector.tensor_copy(bc, bc_ps)
                state["bc"] = bc
            def back():
                bc = state["bc"]
                nc.vector.tensor_mul(ct[0:64, :], cxs[0:64, 0:512], bc[0:64, :])
                nc.vector.tensor_mul(ct[64:128, :], cxs[64:128, 512:1024],
                                     bc[64:128, :])
            return p, t, bcast, back

        # ---- attention for one pair, pulling filler units between exp groups ----
        def attention(p, qt, kt, filler):
            # S^T for both heads, row-tiled (contraction d=64 each).
            # Diagonal-crossing blocks only compute the live q-range [qlo:512).
            def do_scores(t, j):
                off = j * 128 - t * 512
                qlo = max(off, 0)
                sps = ps_s.tile([128, 1024], f32, tag="s")
                nc.tensor.matmul(
                    sps[:, qlo:512], kt[0:64, j * 128 : (j + 1) * 128],
                    qt[0:64, t * 512 + qlo : (t + 1) * 512],
                    start=True, stop=True, tile_position=(0, 0))
                nc.tensor.matmul(
                    sps[:, 512 + qlo : 1024], kt[64:128, j * 128 : (j + 1) * 128],
                    qt[64:128, t * 512 + qlo : (t + 1) * 512],
                    start=True, stop=True, tile_position=(64, 0))
                return sps, qlo

            carry = [do_scores(0, 0)]  # software-pipeline: scores run one block ahead
            for t in range(NQT):
                nk = 4 * (t + 1)
                norms = list(pending_norm)
                pending_norm.clear()
                cx = ps_ctx.tile([128, 1024], f32, tag="ctx")
                for j in range(nk):
                    if j == 2:
                        for _, _, bcast, _ in norms:
                            bcast()
                    if j == 3:
                        for pp, tt, _, bk in norms:
                            bk()
                            if pp % 2 == 1:
                                dest = (reserve if (pp == 1 and tt == NQT - 1)
                                        else filler)
                                for qq in range(4):
                                    for half in range(2):
                                        dest.append(outproj_unit(
                                            pp // 2, tt, qq, half))
                    sps, qlo = carry.pop(0)
                    off = j * 128 - t * 512
                    if j + 1 < nk:
                        carry.append(do_scores(t, j + 1))
                    # exp(scale * S^T); diagonal blocks split per head to
                    # touch only the live q-range
                    ph = p_phat.tile([128, 1024], BF, tag="phat")
                    if qlo == 0:
                        nc.scalar.activation(ph, sps, EXP, scale=SCALE)
                    else:
                        nc.scalar.activation(ph[:, qlo:512], sps[:, qlo:512],
                                             EXP, scale=SCALE)
                        nc.scalar.activation(ph[:, 512 + qlo : 1024],
                                             sps[:, 512 + qlo : 1024],
                                             EXP, scale=SCALE)
                    # causal zeroing on diagonal-crossing blocks (k0 > q0 part)
                    if off + 127 > 0:  # block crosses the diagonal
                        for h in range(2):
                            nc.gpsimd.affine_select(
                                out=ph[:, h * 512 + qlo : (h + 1) * 512],
                                in_=ph[:, h * 512 + qlo : (h + 1) * 512],
                                compare_op=mybir.AluOpType.is_ge,
                                fill=0.0, base=qlo - off,
                                pattern=[[1, 512 - qlo]], channel_multiplier=-1)
                    # ctx'^T accumulation. A: [V_A|ones] -> parts 0:64 (denomA
                    # at 64).  B: [ones|zeros|V_B] -> denomB at part 0, ctx'B
                    # at parts 64:127.  One accumulation group per bank.
                    st, sp = (j == 0), (j == nk - 1)
                    assert v_sb[j] is not None, f"V tile {j} not emitted yet"
                    vt = v_sb[j]
                    nc.tensor.matmul(cx[0:65, qlo:512], vt[:, p, 0:65],
                                     ph[:, qlo:512], start=st, stop=sp)
                    nc.tensor.matmul(cx[:, 512 + qlo : 1024], vt[:, p, 68:196],
                                     ph[:, 512 + qlo : 1024], start=st, stop=sp)
                    if j == nk - 1 and t + 1 < NQT:
                        # next tile's first scores run on PE while the ctx'
                        # psum is evicted on DVE
                        carry.append(do_scores(t + 1, 0))
                    if filler and j >= (3 if norms else 1):
                        filler.pop(0)()
                # Evict unnormalized ctx' to SBUF so the psum banks free, and
                # kick off the denominators' reciprocals immediately.  The
                # broadcast + muls are deferred into the NEXT q-tile iteration.
                cxs = p_cxs.tile([128, 1024], f32, tag="cxs")
                nc.vector.tensor_copy(cxs[0:65, 0:512], cx[0:65, 0:512])
                nc.vector.tensor_copy(cxs[:, 512:1024], cx[:, 512:1024])
                pending_norm.append(make_norm(p, t, cxs))
                if filler:
                    filler.pop(0)()

        # ================= emission schedule =================
        # Minimal head: V tiles 0..3 and pair-0 Q/K for tt=0 only, then start
        # attention; the rest of pair-0's projections interleave with the
        # remaining V tiles as deadline-ordered fillers.
        for j in range(4 * 1):
            v_unit(j)()
        qk_unit(0, "q", w0q, 0)()
        qk_unit(0, "k", w0k, 0)()

        for p in range(NPAIR):
            filler = []
            if p == NPAIR - 1:
                filler.extend(reserve)
                reserve.clear()
            if p == 0:
                for tt in range(1, NQT):
                    filler.append(qk_unit(0, "q", w0q, tt))
                    filler.append(qk_unit(0, "k", w0k, tt))
                    for j in range(4 * tt, 4 * tt + 4):
                        filler.append(v_unit(j))
            if p + 1 < NPAIR:
                wq_t, wk_t = load_wqk(p + 1)
                for tt in range(NQT):
                    filler.append(qk_unit(p + 1, "q", wq_t, tt))
                    filler.append(qk_unit(p + 1, "k", wk_t, tt))
            wo_tiles[p] = load_wo(p)
            attention(p, qkT[("q", p)], qkT[("k", p)], filler)
            for u in filler:  # drain any leftovers
                u()
            qkT.pop(("q", p)), qkT.pop(("k", p))
        # tail: last tile's normalize + its out-projection
        for pp, tt, bcast, bk in pending_norm:
            bcast(); bk()
            for k, (qq, half) in enumerate(
                    (q, h) for q in range(4) for h in range(2)):
                outproj_unit(pp // 2, tt, qq, half, on_act=(k % 2 == 1))()
        pending_norm.clear()

    nc.compile()
    return nc


_NC_CACHE = {}


def _get_nc(T):
    if T not in _NC_CACHE:
        _NC_CACHE[T] = build_kernel(T)
    return _NC_CACHE[T]


def make_in_maps(x, W_q, W_k, W_v, W_o):
    bf = ml_dtypes.bfloat16
    B, T, _ = x.shape
    in_maps = []
    for c in range(8):
        b, g = c // 2, c % 2
        cols = slice(g * HG, (g + 1) * HG)
        in_maps.append({
            "xT": np.ascontiguousarray(x[b].T.astype(bf)),
            "wq": np.ascontiguousarray(W_q[:, cols].astype(bf)),
            "wk": np.ascontiguousarray(W_k[:, cols].astype(bf)),
            "wv": np.ascontiguousarray(W_v[:, cols].astype(bf)),
            "wo": np.ascontiguousarray(W_o[cols, :].astype(bf)),
        })
    return in_maps


def kernel(x, W_q, W_k, W_v, W_o, b_o):
    x = np.asarray(x, dtype=np.float32)
    B, T, C_ = x.shape
    nc = _get_nc(T)
    in_maps = make_in_maps(x, np.asarray(W_q), np.asarray(W_k), np.asarray(W_v),
                           np.asarray(W_o))
    res = run_bass_kernel_spmd(nc, in_maps, core_ids=list(range(8)))
    out = np.empty((B, T, C_), dtype=np.float32)
    for b in range(B):
        pa = np.asarray(res.results[2 * b]["out"]).astype(np.float32).sum(axis=0)
        pb = np.asarray(res.results[2 * b + 1]["out"]).astype(np.float32).sum(axis=0)
        out[b] = pa + pb + np.asarray(b_o, dtype=np.float32)[None, :]
    return out


# revision 28
# speedup vs baseline: 1.0064x; 1.0064x over previous
"""Multi-head causal attention (B=4, T=2048, C=1024, H=16) on 8 TRN2 NeuronCores.

Sharding: core c handles batch b=c//2 and head-group g=c%2 (8 heads = 4 pairs).
Per core: QKV projections for its 512 feature columns, causal attention for its
8 heads, partial out-projection. Host sums the two head-group partials per batch
and adds b_o.

All matmul operands are bf16 (f32 PSUM accumulation): 1 cyc/row moving, FWL on
LDWEIGHTS, half the SBUF/DMA of fp32. Softmax normalize: head-B's V' stationary
is padded to 128 cols ([ones | 63 zeros | V_B]) so ctx'B lands on PSUM
partitions 64:127 (denomB at part 0) while ctx'A ([V_A | ones]) lands at 0:64
(denomA at 64); the denominator rows are spread over 64 lanes via gpsimd-queue
DMAs, reciprocated once on DVE (bf16), and a K=33 selector matmul broadcasts
recipA/recipB across psum partitions 0:63/64:127.  The Scalar queue carries
ONLY exp ACTIVATEs (anything else queued there stalls the whole pipeline), and
scores are emitted one block ahead of ctx so the PE computes next-block scores
under the current exp.  Out-projection accumulates pair-groups (0+1, 2+3) in
PSUM, halving the psum-evacuation casts and the output DMA.
"""
import sys
import numpy as np
import ml_dtypes
from contextlib import ExitStack

sys.path.insert(0, "/opt/trn_rl_repo")

import concourse.bass as bass
import concourse.tile as tile
from concourse import bacc, mybir
from concourse.bass_utils import run_bass_kernel_spmd

f32 = mybir.dt.float32
f32r = mybir.dt.float32r
BF = mybir.dt.bfloat16
EXP = mybir.ActivationFunctionType.Exp
LN = mybir.ActivationFunctionType.Ln

C = 1024          # model dim
HG = 512          # per-core head-group feature width (8 heads x 64)
D = 64            # head dim
NPAIR = 4         # head pairs per core
NCC = C // 128    # contraction chunks (8)
SCALE = 0.125     # 1/sqrt(D)
VSLOT = 196       # V' slot: [V_A 0:64 | onesA 64:65 | pad | onesB 68:69 | zeros 69:132 | V_B 132:196]


def build_kernel(T):
    """Emit the per-core Bass program. T = sequence length (multiple of 512)."""
    NQT = T // 512    # q tiles of 512
    NKT = T // 128    # k tiles of 128

    nc = bacc.Bacc("TRN2", target_bir_lowering=False, debug=False, num_devices=8)

    xT = nc.dram_tensor("xT", [C, T], BF, kind="ExternalInput").ap()
    wq = nc.dram_tensor("wq", [C, HG], BF, kind="ExternalInput").ap()
    wk = nc.dram_tensor("wk", [C, HG], BF, kind="ExternalInput").ap()
    wv = nc.dram_tensor("wv", [C, HG], BF, kind="ExternalInput").ap()
    wo = nc.dram_tensor("wo", [HG, C], BF, kind="ExternalInput").ap()
    out = nc.dram_tensor("out", [NPAIR // 2, T, C], BF, kind="ExternalOutput").ap()

    with tile.TileContext(nc) as tc, ExitStack() as ctx:
        # ---- SBUF pools (bytes/partition noted) ----
        p_xt = ctx.enter_context(tc.tile_pool(name="xt", bufs=1))            # 32K
        p_w = ctx.enter_context(tc.tile_pool(name="w", bufs=2))              # 2x2K
        p_wv = ctx.enter_context(tc.tile_pool(name="wv", bufs=1))            # 8K
        p_wo = ctx.enter_context(tc.tile_pool(name="wo", bufs=4))            # 4x2K
        p_qk = ctx.enter_context(tc.tile_pool(name="qk", bufs=4))            # 4x4K=16K
        p_v = ctx.enter_context(tc.tile_pool(name="v", bufs=4))              # 4x6.2K=25K
        p_phat = ctx.enter_context(tc.tile_pool(name="phat", bufs=4))        # 4x2K
        p_ctxT = ctx.enter_context(tc.tile_pool(name="ctxT", bufs=12))       # 12x1K
        p_cxs = ctx.enter_context(tc.tile_pool(name="cxs", bufs=3))          # 3x4K
        p_bc = ctx.enter_context(tc.tile_pool(name="bc", bufs=2))            # 2x2K
        p_small = ctx.enter_context(tc.tile_pool(name="small", bufs=2))      # recips
        p_ostg = ctx.enter_context(tc.tile_pool(name="ostg", bufs=5))        # 5x1K
        p_ones = ctx.enter_context(tc.tile_pool(name="ones", bufs=1))
        # ---- PSUM pools: 4 + 2 + 2 = 8 banks ----
        ps_s = ctx.enter_context(tc.tile_pool(name="ps_s", bufs=2, space="PSUM"))    # [128,1024] x2
        ps_ctx = ctx.enter_context(tc.tile_pool(name="ps_ctx", bufs=1, space="PSUM"))
        ps_mm = ctx.enter_context(tc.tile_pool(name="ps_mm", bufs=2, space="PSUM"))

        # ---- constants + bulk loads ----
        ones_f = p_ones.tile([128, 1], f32)
        nc.vector.memset(ones_f, 1.0)
        # Selector stationary for the reciprocal broadcast: row 0 routes the
        # moving tile's row 0 (recipA) to psum parts 0:63, row 32 routes
        # recipB to parts 64:127; rows 1:31 are zero (matmul K=33).
        ones33 = p_ones.tile([33, 128], BF, tag="ones33")
        nc.vector.memset(ones33, 0.0)
        nc.vector.memset(ones33[0:1, 0:64], 1.0)
        nc.vector.memset(ones33[32:33, 64:128], 1.0)

        # HAM warm-up: ~8us of dummy matmuls during the input DMAs so the
        # PE clock is at 2.4GHz when real work starts.
        warm = p_ostg.tile([128, 512], BF, tag="ostg")
        nc.vector.memset(warm, 0.0)
        wps = ps_mm.tile([128, 512], f32, tag="mm")
        for i in range(28):
            nc.tensor.matmul(wps, warm[:, 0:128], warm,
                             start=(i == 0), stop=(i == 27))

        def load_wqk(p):
            """[128, 8, 128] tile: cc-chunks of W{q,k}[:, p*128:(p+1)*128]."""
            tq = p_w.tile([128, NCC, 128], BF, tag="wq")
            tk = p_w.tile([128, NCC, 128], BF, tag="wk")
            nc.sync.dma_start(
                tq, wq[:, p * 128 : (p + 1) * 128].rearrange("(cc p) f -> p cc f", p=128))
            nc.sync.dma_start(
                tk, wk[:, p * 128 : (p + 1) * 128].rearrange("(cc p) f -> p cc f", p=128))
            return tq, tk

        def load_wo(p):
            t_ = p_wo.tile([128, C], BF, tag="wo")
            nc.sync.dma_start(t_, wo[p * 128 : (p + 1) * 128, :])
            return t_

        # weights first (small), then x in 4 chunk-pair DMAs so the V/QK
        # projection matmuls pipeline behind the arriving chunks; the pair-0
        # weight loads ride idle queues so the x chunks start immediately
        tq0 = p_w.tile([128, NCC, 128], BF, tag="wq")
        tk0 = p_w.tile([128, NCC, 128], BF, tag="wk")
        nc.scalar.dma_start(
            tq0, wq[:, 0:128].rearrange("(cc p) f -> p cc f", p=128))
        nc.gpsimd.dma_start(
            tk0, wk[:, 0:128].rearrange("(cc p) f -> p cc f", p=128))
        w0q, w0k = tq0, tk0
        wv_sb = p_wv.tile([128, NCC, HG], BF)
        nc.scalar.dma_start(wv_sb, wv.rearrange("(cc p) f -> p cc f", p=128))
        xt_all = p_xt.tile([128, NCC, T], BF, tag="xt")
        for i, eng in zip(range(4), (nc.sync, nc.gpsimd, nc.sync, nc.gpsimd)):
            eng.dma_start(
                xt_all[:, 2 * i : 2 * i + 2, :],
                xT[2 * i * 128 : (2 * i + 2) * 128, :].rearrange(
                    "(cc p) t -> p cc t", p=128))
        xt = [xt_all[:, cc, :] for cc in range(NCC)]
        # x chunk-pairs land as (0,1)+(2,3) on sync and (4,5)+(6,7) on gpsimd;
        # consume contraction chunks in arrival order (psum accumulation is
        # order-independent) so early chunks aren't stuck behind late ones
        CC_ORDER = [0, 1, 4, 5, 2, 3, 6, 7]

        # ---- filler unit generators (PE work to hide under ACT-bound attention) ----
        v_groups = [None] * (NKT // 4)   # [128, 4, NPAIR, VSLOT] tiles, 4 k-tiles each

        def v_tile(j):
            g = v_groups[j // 4]
            assert g is not None, f"V group {j // 4} not emitted yet"
            return g[:, j % 4]

        v_sb = [None] * NKT

        def v_unit(j):
            def emit():
                ps = ps_mm.tile([128, HG], f32, tag="mm")
                for k, cc in enumerate(CC_ORDER):
                    nc.tensor.matmul(
                        ps, xt[cc][:, j * 128 : (j + 1) * 128],
                        wv_sb[:, cc, :], start=(k == 0), stop=(k == NCC - 1))
                if j % 4 == 0:
                    g = p_v.tile([128, 4, NPAIR, VSLOT], BF, tag="v",
                                 name=f"vg{j // 4}")
                    v_groups[j // 4] = g
                    # constant cols for the whole group in 3 strided DVE ops
                    nc.vector.memset(g[:, :, :, 69:132], 0.0)
                    nc.vector.tensor_copy(
                        g[:, :, :, 64:65], ones_f.to_broadcast([128, 4, NPAIR, 1]))
                    nc.vector.tensor_copy(
                        g[:, :, :, 68:69], ones_f.to_broadcast([128, 4, NPAIR, 1]))
                g = v_groups[j // 4]
                # one strided copy scatters all 4 pairs' V_A/V_B blocks:
                # dest free dims [pair(196), head(132), 64]
                dst = bass.AP(
                    tensor=g.tensor, offset=g.offset + (j % 4) * NPAIR * VSLOT,
                    ap=[list(g.ap[0]), [VSLOT, NPAIR], [132, 2], [1, 64]])
                nc.vector.tensor_copy(
                    dst, ps.rearrange("p (a h d) -> p a h d", a=NPAIR, h=2))
                v_sb[j] = v_tile(j)
            return emit

        qkT = {}   # (('q'|'k'), pair) -> [128, T] bf16 tile

        def qk_unit(p, which, wtile, tt):
            def emit():
                key = (which, p)
                if key not in qkT:
                    qkT[key] = p_qk.tile([128, T], BF, tag="qk", name=f"qk_{which}{p}")
                ps = ps_mm.tile([128, 512], f32, tag="mm")
                for k, cc in enumerate(CC_ORDER):
                    nc.tensor.matmul(
                        ps, wtile[:, cc, :], xt[cc][:, tt * 512 : (tt + 1) * 512],
                        start=(k == 0), stop=(k == NCC - 1))
                nc.vector.tensor_copy(qkT[key][:, tt * 512 : (tt + 1) * 512], ps)
            return emit

        ctxT_store = {}  # (p, t) -> [128, 512] bf16 tile

        def outproj_unit(g2, t, qq, half, on_act=False):
            def emit():
                stg = p_ostg.tile([128, 512], BF, tag="ostg")
                ps = ps_mm.tile([128, 512], f32, tag="mm")
                for p in (2 * g2, 2 * g2 + 1):
                    ct = ctxT_store[(p, t)]
                    nc.tensor.matmul(
                        ps, ct[:, qq * 128 : (qq + 1) * 128],
                        wo_tiles[p][:, half * 512 : (half + 1) * 512],
                        start=(p == 2 * g2), stop=(p == 2 * g2 + 1))
                if on_act:  # tail only: Scalar is idle once the exps are done
                    nc.scalar.copy(stg, ps)
                else:
                    nc.vector.tensor_copy(stg, ps)
                if qq == 3 and half == 1:
                    ctxT_store.pop((2 * g2, t))
                    ctxT_store.pop((2 * g2 + 1, t))
                nc.sync.dma_start(
                    out[g2, t * 512 + qq * 128 : t * 512 + (qq + 1) * 128,
                        half * 512 : (half + 1) * 512], stg)
            return emit

        pending_norm = []
        reserve = []   # outproj units held back for pair 3's filler-starved start
        wo_tiles = {}

        def make_norm(p, t, cxs):
            ct = p_ctxT.tile([128, 512], BF, tag="ctxT", name=f"ct_{p}_{t}")
            ctxT_store[(p, t)] = ct
            # reciprocal rows now, while the next tile's scores run: spread
            # the two denominator rows over 64 partitions (gpsimd-queue DMAs,
            # keeping Scalar free for exp), one DVE reciprocal, gather the
            # bf16 recips straight into the selector-matmul moving tile.
            rr = p_small.tile([33, 512], BF, tag="rr")
            nc.vector.memset(rr[0:32, :], 0.0)
            sc = p_small.tile([64, 16], f32, tag="sc")
            scb = p_small.tile([64, 16], BF, tag="scb")
            nc.gpsimd.dma_start(sc[0:32, :], cxs[64:65, 0:512])
            nc.gpsimd.dma_start(sc[32:64, :], cxs[0:1, 512:1024])
            with nc.allow_low_precision("softmax 1/denom rows in bf16"):
                nc.vector.reciprocal(scb, sc)
            nc.gpsimd.dma_start(rr[0:1, :], scb[0:32, :])
            nc.gpsimd.dma_start(rr[32:33, :], scb[32:64, :])
            state = {}
            def bcast():
                # PE broadcast: recipA -> psum parts 0:63, recipB -> 64:127
                bc_ps = ps_mm.tile([128, 512], f32, tag="mm")
                nc.tensor.matmul(bc_ps, ones33, rr, start=True, stop=True)
                bc = p_bc.tile([128, 512], f32, tag="bc")
                nc.vector.tensor_copy(bc, bc_ps)
                state["bc"] = bc
            def back():
                bc = state["bc"]
                nc.vector.tensor_mul(ct[0:64, :], cxs[0:64, 0:512], bc[0:64, :])
                nc.vector.tensor_mul(ct[64:128, :], cxs[64:128, 512:1024],
                                     bc[64:128, :])
            return p, t, bcast, back

        # ---- attention for one pair, pulling filler units between exp groups ----
        def attention(p, qt, kt, filler):
            # S^T for both heads, row-tiled (contraction d=64 each).
            # Diagonal-crossing blocks only compute the live q-range [qlo:512).
            def do_scores(t, j):
                off = j * 128 - t * 512
                qlo = max(off, 0)
                sps = ps_s.tile([128, 1024], f32, tag="s")
                nc.tensor.matmul(
                    sps[:, qlo:512], kt[0:64, j * 128 : (j + 1) * 128],
                    qt[0:64, t * 512 + qlo : (t + 1) * 512],
                    start=True, stop=True, tile_position=(0, 0))
                nc.tensor.matmul(
                    sps[:, 512 + qlo : 1024], kt[64:128, j * 128 : (j + 1) * 128],
                    qt[64:128, t * 512 + qlo : (t + 1) * 512],
                    start=True, stop=True, tile_position=(64, 0))
                return sps, qlo

            carry = [do_scores(0, 0)]  # software-pipeline: scores run one block ahead
            for t in range(NQT):
                nk = 4 * (t + 1)
                norms = list(pending_norm)
                pending_norm.clear()
                cx = ps_ctx.tile([128, 1024], f32, tag="ctx")
                for j in range(nk):
                    if j == 2:
                        for _, _, bcast, _ in norms:
                            bcast()
                    if j == 3:
                        for pp, tt, _, bk in norms:
                            bk()
                            if pp % 2 == 1:
                                dest = (reserve if (pp == 1 and tt == NQT - 1)
                                        else filler)
                                for qq in range(4):
                                    for half in range(2):
                                        dest.append(outproj_unit(
                                            pp // 2, tt, qq, half))
                    sps, qlo = carry.pop(0)
                    off = j * 128 - t * 512
                    if j + 1 < nk:
                        carry.append(do_scores(t, j + 1))
                    # exp(scale * S^T); diagonal blocks split per head to
                    # touch only the live q-range
                    ph = p_phat.tile([128, 1024], BF, tag="phat")
                    if qlo == 0:
                        nc.scalar.activation(ph, sps, EXP, scale=SCALE)
                    else:
                        nc.scalar.activation(ph[:, qlo:512], sps[:, qlo:512],
                                             EXP, scale=SCALE)
                        nc.scalar.activation(ph[:, 512 + qlo : 1024],
                                             sps[:, 512 + qlo : 1024],
                                             EXP, scale=SCALE)
                    # causal zeroing on diagonal-crossing blocks (k0 > q0 part)
                    if off + 127 > 0:  # block crosses the diagonal
                        for h in range(2):
                            nc.gpsimd.affine_select(
                                out=ph[:, h * 512 + qlo : (h + 1) * 512],
                                in_=ph[:, h * 512 + qlo : (h + 1) * 512],
                                compare_op=mybir.AluOpType.is_ge,
                                fill=0.0, base=qlo - off,
                                pattern=[[1, 512 - qlo]], channel_multiplier=-1)
                    # ctx'^T accumulation. A: [V_A|ones] -> parts 0:64 (denomA
                    # at 64).  B: [ones|zeros|V_B] -> denomB at part 0, ctx'B
                    # at parts 64:127.  One accumulation group per bank.
                    st, sp = (j == 0), (j == nk - 1)
                    assert v_sb[j] is not None, f"V tile {j} not emitted yet"
                    vt = v_sb[j]
                    nc.tensor.matmul(cx[0:65, qlo:512], vt[:, p, 0:65],
                                     ph[:, qlo:512], start=st, stop=sp)
                    nc.tensor.matmul(cx[:, 512 + qlo : 1024], vt[:, p, 68:196],
                                     ph[:, 512 + qlo : 1024], start=st, stop=sp)
                    if j == nk - 1 and t + 1 < NQT:
                        # next tile's first scores run on PE while the ctx'
                        # psum is evicted on DVE
                        carry.append(do_scores(t + 1, 0))
                    if filler and j >= (3 if norms else 1):
                        filler.pop(0)()
                # Evict unnormalized ctx' to SBUF so the psum banks free, and
                # kick off the denominators' reciprocals immediately.  The
                # broadcast + muls are deferred into the NEXT q-tile iteration.
                cxs = p_cxs.tile([128, 1024], f32, tag="cxs")
                nc.vector.tensor_copy(cxs[0:65, 0:512], cx[0:65, 0:512])
                nc.vector.tensor_copy(cxs[:, 512:1024], cx[:, 512:1024])
                pending_norm.append(make_norm(p, t, cxs))
                if filler:
                    filler.pop(0)()

        # ================= emission schedule =================
        # Minimal head: V tiles 0..3 and pair-0 Q/K for tt=0 only, then start
        # attention; the rest of pair-0's projections interleave with the
        # remaining V tiles as deadline-ordered fillers.
        for j in range(4 * 1):
            v_unit(j)()
        qk_unit(0, "q", w0q, 0)()
        qk_unit(0, "k", w0k, 0)()

        for p in range(NPAIR):
            filler = []
            if p == NPAIR - 1:
                filler.extend(reserve)
                reserve.clear()
            if p == 0:
                for tt in range(1, NQT):
                    filler.append(qk_unit(0, "q", w0q, tt))
                    filler.append(qk_unit(0, "k", w0k, tt))
                    for j in range(4 * tt, 4 * tt + 4):
                        filler.append(v_unit(j))
            if p + 1 < NPAIR:
                wq_t, wk_t = load_wqk(p + 1)
                for tt in range(NQT):
                    filler.append(qk_unit(p + 1, "q", wq_t, tt))
                    filler.append(qk_unit(p + 1, "k", wk_t, tt))
            wo_tiles[p] = load_wo(p)
            attention(p, qkT[("q", p)], qkT[("k", p)], filler)
            for u in filler:  # drain any leftovers
                u()
            qkT.pop(("q", p)), qkT.pop(("k", p))
        # tail: last tile's normalize + its out-projection
        for pp, tt, bcast, bk in pending_norm:
            bcast(); bk()
            for k, (qq, half) in enumerate(
                    (q, h) for q in range(4) for h in range(2)):
                outproj_unit(pp // 2, tt, qq, half, on_act=(k % 2 == 1))()
        pending_norm.clear()

    nc.compile()
    return nc


_NC_CACHE = {}


def _get_nc(T):
    if T not in _NC_CACHE:
        _NC_CACHE[T] = build_kernel(T)
    return _NC_CACHE[T]


def make_in_maps(x, W_q, W_k, W_v, W_o):
    bf = ml_dtypes.bfloat16
    B, T, _ = x.shape
    in_maps = []
    for c in range(8):
        b, g = c // 2, c % 2
        cols = slice(g * HG, (g + 1) * HG)
        in_maps.append({
            "xT": np.ascontiguousarray(x[b].T.astype(bf)),
            "wq": np.ascontiguousarray(W_q[:, cols].astype(bf)),
            "wk": np.ascontiguousarray(W_k[:, cols].astype(bf)),
            "wv": np.ascontiguousarray(W_v[:, cols].astype(bf)),
            "wo": np.ascontiguousarray(W_o[cols, :].astype(bf)),
        })
    return in_maps


def kernel(x, W_q, W_k, W_v, W_o, b_o):
    x = np.asarray(x, dtype=np.float32)
    B, T, C_ = x.shape
    nc = _get_nc(T)
    in_maps = make_in_maps(x, np.asarray(W_q), np.asarray(W_k), np.asarray(W_v),
                           np.asarray(W_o))
    res = run_bass_kernel_spmd(nc, in_maps, core_ids=list(range(8)))
    out = np.empty((B, T, C_), dtype=np.float32)
    for b in range(B):
        pa = np.asarray(res.results[2 * b]["out"]).astype(np.float32).sum(axis=0)
        pb = np.asarray(res.results[2 * b + 1]["out"]).astype(np.float32).sum(axis=0)
        out[b] = pa + pb + np.asarray(b_o, dtype=np.float32)[None, :]
    return out
